# revision 29
# baseline (speedup 1.0000x reference)
"""DMM (deep markov model) Trainium2 kernel — gate-major formulation.

Reference: hybrid embedder -> backward LSTM over T -> recurrent VAE
(inference encoder / sampler / decoder / reconstructor); only the LAST
timestep's reconstruction sample [B, D] is returned.

Strategy (8 NeuronCores, pure batch data-parallel, B_local = 64):
  * Everything on-chip is TRANSPOSED (gate-major): activations are [dim, batch]
    tiles, so the recurrent states h^T / z^T come straight out of the
    elementwise ops and feed the next matmul as the moving operand — no
    per-step transposes at all.
  * Matmul operands are fp16 (weights stationary with fast-weight-load);
    PSUM accumulation and the recurrent cell state stay fp32.
  * igates (input projections) for 2 steps at a time are computed by an
    efficient [128]-wide matmul; the per-step recurrent matmul accumulates
    into the same PSUM region.
  * sigmoid(x) = (tanh(x/2)+1)/2; the g-gate columns are pre-scaled by 2 so a
    single Tanh(scale=0.5) covers all 4 gates; gate order is permuted to
    [f, i, g, o] and the pre-activations live in three separate PSUM tiles
    (f | i,g | o) so the f tanh that starts the serial cell chain waits on
    only 4 of the 16 recurrent matmuls.  tanh and exp share one ACT table set.
  * h is carried as h2 = 2h (recurrent/encoder weights pre-scaled), the cell
    update is 4 fused scalar_tensor_tensor ops.
  * Embedding lookup is a one-hot matmul against the constant-folded
    emb @ Ke + b table.  Decoder/reconstructor only at the final step.

kernel(**inputs) takes FULL inputs, shards batch over 8 cores, returns the
full [512, 256] float32 output.
"""

import sys

sys.path.insert(0, "/opt/trn_rl_repo")

import numpy as np

import concourse.bass as bass
import concourse.bacc as bacc
import concourse.tile as tile
from concourse import mybir
from concourse.bass_utils import run_bass_kernel_spmd

B, T, H, E, F, V = 512, 128, 256, 128, 128, 64
D = E + F
NC = 8
BL = B // NC  # 64
FD = mybir.dt.float32
FH = mybir.dt.float16

# gate order [i, f, g, o] -> [f, i, g, o]
_PERM = np.concatenate([np.arange(256, 512), np.arange(0, 256),
                        np.arange(512, 768), np.arange(768, 1024)])


def _prep_inputs(events, features, emb, lstm_k, lstm_rk, lstm_b,
                 inf_Wm, inf_bm, inf_Wv, inf_bv,
                 dec_Wm, dec_bm, dec_Wv, dec_bv,
                 z0, eps_inf, eps_rec):
    f16, f32 = np.float16, np.float32
    cs = np.ones((4 * H,), f32)
    cs[2 * H:3 * H] = 2.0  # g columns (pre-perm)

    kf2 = (lstm_k[E:] * cs[None, :])[:, _PERM].astype(f16)          # [128,1024]
    kf2 = np.ascontiguousarray(kf2.reshape(F, 8, 128))
    m1p = (np.asarray(emb, np.float64) @ np.asarray(lstm_k[:E] * cs[None, :], np.float64)
           + np.asarray(lstm_b * cs, np.float64))
    m1p = np.ascontiguousarray(m1p[:, _PERM].astype(f16).reshape(V, 8, 128))
    rk2 = ((lstm_rk * cs[None, :] * 0.5)[:, _PERM]).astype(f16)     # [256,1024]
    rk2 = np.ascontiguousarray(rk2.reshape(2, 128, 8, 128).transpose(1, 0, 2, 3))
    wmv = np.concatenate([inf_Wm, inf_Wv], axis=1).astype(f32)      # [512,512]
    wmv[:H] *= 0.5  # g rows see h2 = 2h
    wmv = np.ascontiguousarray(wmv.astype(f16).reshape(4, 128, 4, 128)
                               .transpose(1, 0, 2, 3))              # [128,4,4,128]
    bmv = np.concatenate([inf_bm, inf_bv])[None, :].astype(f16)     # [1,512]
    # per-partition bias columns for the gate-major encoder: [128, 4] fp32,
    # cols 0,1 = bm halves (added via STT), cols 2,3 = 0.5*bv halves (folded
    # into the exp activation's bias port: exp(0.5*zlv + 0.5*bv)).
    bmh = np.concatenate([inf_bm, 0.5 * inf_bv]).astype(f32).reshape(4, 128).T
    bmh = np.ascontiguousarray(bmh)                                 # [128,4]
    decw = np.concatenate([dec_Wm, dec_Wv], axis=1).astype(f16)     # [256,512]
    decw = np.ascontiguousarray(decw.reshape(2, 128, 4, 128).transpose(1, 0, 2, 3))
    decb = np.concatenate([dec_bm, dec_bv])[None, :].astype(f16)    # [1,512]
    ones = np.ones((1, BL), f16)

    shared = dict(kf2=kf2, m1p=m1p, rk2=rk2, wmv=wmv, bmv=bmv, bmh=bmh,
                  decw=decw, decb=decb, ones=ones)
    per_core = []
    ev = np.asarray(events)
    for c in range(NC):
        bc = slice(c * BL, (c + 1) * BL)
        featr = np.asarray(features[bc][:, ::-1], f32)               # [BL,T,F]
        xfT = np.ascontiguousarray(featr.transpose(2, 1, 0).reshape(F, T * BL)).astype(f16)
        evr = ev[bc][:, ::-1].T.reshape(-1)
        oneh = (np.arange(V)[:, None] == evr[None, :]).astype(f16)   # [64,T*BL]
        z0T = np.ascontiguousarray(np.asarray(z0[bc], f32).T.reshape(2, 128, BL)
                                   .transpose(1, 0, 2)).astype(f16)  # [128,2,64]
        epsT = np.ascontiguousarray(
            np.asarray(eps_inf[:, bc, :], f32).reshape(T, BL, 2, 128)
            .transpose(3, 0, 2, 1).reshape(128, T * 2 * BL)).astype(f16)
        epsrT = np.ascontiguousarray(
            np.asarray(eps_rec[T - 1, bc, :], f32).reshape(BL, 2, 128)
            .transpose(2, 1, 0)).astype(f16)                         # [128,2,64]
        m = dict(xfT=xfT, oneh=oneh, z0T=z0T, epsT=epsT, epsrT=epsrT)
        m.update(shared)
        per_core.append(m)
    return per_core


_SKIP_DMM = False


def _build_bass():
    nc = bacc.Bacc("TRN2", target_bir_lowering=False, debug=False)
    d = {}
    def din(name, shape, dt=FH):
        d[name] = nc.dram_tensor(name, list(shape), dt, kind="ExternalInput").ap()
    din("xfT", (F, T * BL))
    din("oneh", (V, T * BL))
    din("m1p", (V, 8, 128))
    din("kf2", (F, 8, 128))
    din("rk2", (128, 2, 8, 128))
    din("wmv", (128, 4, 4, 128))
    din("bmv", (1, 4 * 128))
    din("bmh", (128, 4), FD)
    din("decw", (128, 2, 4, 128))
    din("decb", (1, 4 * 128))
    din("z0T", (128, 2, BL))
    din("ones", (1, BL))
    din("epsT", (128, T * 2 * BL))
    din("epsrT", (128, 2, BL))
    out = nc.dram_tensor("out", [128, 2 * BL], FD, kind="ExternalOutput").ap()

    Tanh = mybir.ActivationFunctionType.Tanh
    Exp = mybir.ActivationFunctionType.Exp
    add = mybir.AluOpType.add
    mult = mybir.AluOpType.mult

    from contextlib import ExitStack
    with tile.TileContext(nc) as tc, ExitStack() as es:
        cst = es.enter_context(tc.tile_pool(name="cst", bufs=1))
        epool = es.enter_context(tc.tile_pool(name="eps", bufs=2))
        work = es.enter_context(tc.tile_pool(name="work", bufs=2))
        hzpool = es.enter_context(tc.tile_pool(name="hz", bufs=3))
        pga = es.enter_context(tc.tile_pool(name="pga", bufs=2, space="PSUM"))
        pgb = es.enter_context(tc.tile_pool(name="pgb", bufs=2, space="PSUM"))
        pgc = es.enter_context(tc.tile_pool(name="pgc", bufs=2, space="PSUM"))
        pzl = es.enter_context(tc.tile_pool(name="pzl", bufs=1, space="PSUM"))
        pzm = es.enter_context(tc.tile_pool(name="pzm", bufs=1, space="PSUM"))

        def load(name, shape, dt=FH):
            t_ = cst.tile(list(shape), dt, tag=name)
            nc.sync.dma_start(out=t_, in_=d[name])
            return t_
        xfT = load("xfT", (F, T * BL))
        oneh = load("oneh", (V, T * BL))
        m1p = load("m1p", (V, 8, 128))
        kf2 = load("kf2", (F, 8, 128))
        rk2 = load("rk2", (128, 2, 8, 128))
        wmv = load("wmv", (128, 4, 4, 128))
        decw = load("decw", (128, 2, 4, 128))
        z0T = load("z0T", (128, 2, BL))
        epsrT = load("epsrT", (128, 2, BL))
        bmh = load("bmh", (128, 4), FD)
        def load_row(name, w):
            t_ = cst.tile([128, w], FH, tag=name)
            nc.sync.dma_start(out=t_[0:1, :], in_=d[name])
            return t_
        bmv = load_row("bmv", 4 * 128)
        decb = load_row("decb", 4 * 128)
        onesr = load_row("ones", BL)

        EPC = 8
        eps_t = None
        eps_sl = {}
        c_prev = None
        h2T = None       # h2^T of step t-1 entering iteration t
        h2T_new = None
        zT_prev = z0T    # z^T of step t-2 entering iteration t (DMM lag 1)
        pgt = None

        def dmm_step(h2T_g, zT_g, eps_ap, last=False):
            """Encoder+sampler for one step, gate-major.  Returns zT tile."""
            lv = pzl.tile([128, 2, BL], FD, tag="pzl")
            mu = pzm.tile([128, 2, BL], FD, tag="pzm")
            for j in range(2):  # z_lv^T m-tiles first (E is on the chain)
                for k in range(4):
                    rhs = h2T_g[:, k, :] if k < 2 else zT_g[:, k - 2, :]
                    nc.tensor.matmul(lv[:, j, :], wmv[:, k, 2 + j, :], rhs,
                                     start=(k == 0), stop=(k == 3))
            for j in range(2):
                for k in range(4):
                    rhs = h2T_g[:, k, :] if k < 2 else zT_g[:, k - 2, :]
                    nc.tensor.matmul(mu[:, j, :], wmv[:, k, j, :], rhs,
                                     start=(k == 0), stop=(k == 3))
            # biases enter via the ACT bias port / STT scalar (per-partition)
            Ez = work.tile([128, 2, BL], FH, tag="Ez")
            for j in range(2):
                nc.scalar.activation(Ez[:, j, :], lv[:, j, :], Exp,
                                     bias=bmh[:, 2 + j:3 + j], scale=0.5)
            m1 = work.tile([128, 2, BL], FH, tag="m1")
            nc.vector.tensor_mul(m1, Ez, eps_ap)
            zT_n = hzpool.tile([128, 2, BL], FH, tag="zT")
            for j in range(2):
                nc.vector.scalar_tensor_tensor(zT_n[:, j, :], mu[:, j, :],
                                               bmh[:, j:j + 1], m1[:, j, :],
                                               add, add)
            return zT_n

        for t in range(T):
            if t % EPC == 0:
                eps_t = epool.tile([128, EPC * 2 * BL], FH, tag="eps")
                nc.sync.dma_start(
                    out=eps_t, in_=d["epsT"][:, t * 2 * BL:(t + EPC) * 2 * BL])
            for s in range(min(EPC, T - t)):
                eps_sl[t + s] = eps_t[:, (t + s) % EPC * 2 * BL:((t + s) % EPC + 1) * 2 * BL]

            # ---- igates for steps (t, t+1): three psum group tiles ----
            # groups: A = f (m 0,1), B = i,g (m 2..5), C = o (m 6,7) —
            # separate tiles so the f-gate tanh (start of the serial c-chain)
            # only waits on group A's matmuls, not all 16.
            if t % 2 == 0:
                ga = pga.tile([128, 2, 2 * BL], FD, tag="pga")
                gb = pgb.tile([128, 4, 2 * BL], FD, tag="pgb")
                gc = pgc.tile([128, 2, 2 * BL], FD, tag="pgc")
                groups = [(ga, 0, 2), (gb, 2, 6), (gc, 6, 8)]
                for gt_, m0, m1_ in groups:
                    for m in range(m0, m1_):
                        nc.tensor.matmul(gt_[:, m - m0, :], kf2[:, m, :],
                                         xfT[:, t * BL:(t + 2) * BL],
                                         start=True, stop=False)
                        nc.tensor.matmul(gt_[:, m - m0, :], m1p[:, m, :],
                                         oneh[:, t * BL:(t + 2) * BL],
                                         start=False, stop=(t == 0))
            sl = slice((t % 2) * BL, (t % 2) * BL + BL)

            # ---- recurrent matmul for step t (skipped at t=0: h=0) ----
            if h2T is not None:
                for gt_, m0, m1_ in groups:
                    for m in range(m0, m1_):
                        for k in range(2):
                            nc.tensor.matmul(gt_[:, m - m0, sl], rk2[:, k, m, :],
                                             h2T[:, k, :], start=False,
                                             stop=(k == 1))

            # ---- DMM step t-1 matmuls (lag 1; uses h2T(t-1), zT(t-2)) ----
            if t > 0 and not _SKIP_DMM:
                zT_cur = dmm_step(h2T, zT_prev, eps_sl[t - 1])

            # ---- LSTM elementwise (gate-major [128, n, 64]) ----
            # groups after perm: A = f, B = i,g, C = o; s1 only needs f so the
            # serial c-chain starts as soon as group A's matmuls finish.
            TgA = work.tile([128, 2, BL], FD, tag="TgA")
            nc.scalar.activation(TgA, ga[:, :, sl], Tanh, scale=0.5)
            TgB = work.tile([128, 4, BL], FD, tag="TgB")
            nc.scalar.activation(TgB, gb[:, :, sl], Tanh, scale=0.5)
            TgC = work.tile([128, 2, BL], FD, tag="TgC")
            nc.scalar.activation(TgC, gc[:, :, sl], Tanh, scale=0.5)
            v = work.tile([128, 2, BL], FD, tag="v")
            nc.vector.scalar_tensor_tensor(v, TgB[:, 0:2, :], 1.0, TgB[:, 2:4, :],
                                           add, mult)
            if c_prev is None:
                cpp = v
            else:
                s1 = work.tile([128, 2, BL], FD, tag="s1")
                nc.vector.scalar_tensor_tensor(s1, TgA, 1.0, c_prev,
                                               add, mult)
                cpp = work.tile([128, 2, BL], FD, tag="cpp")
                nc.vector.scalar_tensor_tensor(cpp, s1, 0.5, v, mult, add)
            c_prev = cpp
            TC = work.tile([128, 2, BL], FD, tag="TC")
            nc.scalar.activation(TC, cpp, Tanh, scale=0.5)
            h2T_new = hzpool.tile([128, 2, BL], FH, tag="hT")
            nc.vector.scalar_tensor_tensor(h2T_new, TgC, 1.0, TC,
                                           add, mult)
            if t > 0 and not _SKIP_DMM:
                zT_prev = zT_cur
            h2T = h2T_new

        # ---- final DMM step (t = T-1) + decoder/reconstructor ----
        zT_last = dmm_step(h2T, zT_prev, eps_sl[T - 1])
        lv = pzl.tile([128, 2, BL], FD, tag="pzl")
        mu = pzm.tile([128, 2, BL], FD, tag="pzm")
        for j in range(2):
            nc.tensor.matmul(lv[:, j, :], decb[0:1, (2 + j) * 128:(3 + j) * 128],
                             onesr[0:1, :], start=True, stop=False)
            for k in range(2):
                nc.tensor.matmul(lv[:, j, :], decw[:, k, 2 + j, :],
                                 zT_last[:, k, :], start=False, stop=(k == 1))
        for j in range(2):
            nc.tensor.matmul(mu[:, j, :], decb[0:1, j * 128:(j + 1) * 128],
                             onesr[0:1, :], start=True, stop=False)
            for k in range(2):
                nc.tensor.matmul(mu[:, j, :], decw[:, k, j, :],
                                 zT_last[:, k, :], start=False, stop=(k == 1))
        E2 = work.tile([128, 2, BL], FD, tag="E2")
        nc.scalar.activation(E2, lv, Exp, scale=0.5)
        m2 = work.tile([128, 2, BL], FD, tag="m2")
        nc.vector.tensor_mul(m2, E2, epsrT)
        xtT = work.tile([128, 2, BL], FD, tag="xtT")
        nc.vector.tensor_add(xtT, mu, m2)
        nc.sync.dma_start(out=out, in_=xtT)
    if not nc.is_finalized():
        nc.finalize()
    return nc


_NC_CACHE = None


def kernel(**inputs) -> np.ndarray:
    global _NC_CACHE
    inputs = {k: np.asarray(v) for k, v in inputs.items()}
    per_core = _prep_inputs(**inputs)
    if _NC_CACHE is None:
        _NC_CACHE = _build_bass()
    res = run_bass_kernel_spmd(_NC_CACHE, per_core, list(range(NC)))
    outs = []
    for c in range(NC):
        oT = np.asarray(res.results[c]["out"])          # [128, 2*BL]
        outs.append(oT.reshape(128, 2, BL).transpose(2, 1, 0).reshape(BL, D))
    return np.concatenate(outs, axis=0).astype(np.float32)
